# revision 60
# baseline (speedup 1.0000x reference)
"""Causal self-attention (B=2, T=2048, C=1024, nh=16) on 8 TRN2 NeuronCores.

Sharding: core c -> batch b = c//4, head group g = c%4 (4 heads each).
Each core computes QKV projections for its heads, causal attention, and a
partial output projection (W_proj rows for its heads). The four partials
per batch are summed on the host, which also adds b_proj.

Layouts (per core, hardcoded):
  xt   [128, 8, 2048]    x[b].T tiles:  xt[p, kt, t] = x[b, t, kt*128+p]
  wqk  [128, 8, 4, 128]  W_attn q|k cols for this core's heads
  wv   [128, 8, 256]     W_attn v cols (flat 256 feature cols)
  bqk  [128, 4] f32      b_attn q|k (per-partition bias)
  bv   [128, 256] bf16   b_attn v broadcast across partitions (host-side)
  wp   [128, 2, 1024]    W_proj rows for this core's heads
  out  [2048, 1024] bf16 partial (x[b] @ ... for this head group)

Single software-pipelined pass (vs the old 3-phase structure):
  for tb in 0..3:
    q,k projections for time block tb (W-stationary, [feat, t] layout)
    attention for i-block ib=tb:  S^T = k @ q.T -> exp on ACT -> P^T @ [v|1]
      - PE filler inside the S->exp->PV dependency gaps: v projections
        (x @ Wv, natural [t, feat] layout, feeding vext directly -- no DMA
        transpose), the previous i-block's output projection, and the NEXT
        time block's q/k projections. The PE never idles (keeps the 2.4 GHz
        p-state) and the Scalar-engine exp stream is hidden under PE work.
      - PV matmuls are deferred by two groups (rolling across head
        boundaries) so no PV ever waits on its exp; each head's softmax
        normalize chain (1/l + broadcast + scale) is deferred behind the
        next head's masks so the PE never waits on the DVE.
  drain: output projection of the last i-block (PSUM->SBUF copies split
  across ACT and DVE).

DMA: one sync-engine ring, issue order = need order (per-kt wqk/xt-tb0
pairs first so the first matmul starts ~9us in, then wv, xt tb1..3 as
single whole-tensor transfers). Output stores are 2KB-line bf16 tiles;
the host sums the 4 partials per batch and adds b_proj (no bias matmuls
on device).

Baseline from the previous session: 200.3us. This version: ~156us.
"""

import os
import sys

sys.path.insert(0, "/opt/trn_rl_repo")
os.environ.setdefault("MYCRO_LOCAL_CACHE", "1")

import ml_dtypes
import numpy as np

import concourse.bass as bass
import concourse.mybir as mybir
import concourse.tile as tile
from concourse import bacc
from concourse.bass_utils import run_bass_kernel_spmd

B, T, C, NH, HS = 2, 2048, 1024, 16, 64
HPC = 4  # heads per core
N_CORES = 8
KT = C // 128  # 8 contraction tiles over C
TT = T // 128  # 16 tiles over T
IB = T // 512  # 4 i-blocks over T
F32 = mybir.dt.float32

CD = mybir.dt.bfloat16
CD_NP = ml_dtypes.bfloat16

LAST_RESULT = None
_CACHE = {}


def _emit(nc, tc, ctx, aps):
    xt, wqk, wv, bqk, bv, wp, out = (
        aps["xt"], aps["wqk"], aps["wv"], aps["bqk"], aps["bv"], aps["wp"],
        aps["out"],
    )
    Exp = mybir.ActivationFunctionType.Exp

    consts = ctx.enter_context(tc.tile_pool(name="consts", bufs=1))

    # --- persistent SBUF tensors + input DMAs in need-order on one ring ---
    xt_s = consts.tile([128, KT, T], CD, tag="xt")
    wqk_s = consts.tile([128, KT, 4, 128], CD, tag="wqk")
    wv_s = consts.tile([128, KT, 256], CD, tag="wv")
    bqk_s = consts.tile([128, 4], F32, tag="bqk")
    bv_s = consts.tile([128, 256], CD, tag="bv")
    wp_s = consts.tile([128, 2, C], CD, tag="wp")

    # need-order: (wqk, xt-tb0) pairs -> bqk -> wv -> bv -> xt tb1 -> wp ->
    # xt tb2, tb3. The ring drains FIFO at full HBM bandwidth, so the first
    # q-projection chain unblocks ~9us in instead of waiting for everything.
    for kt in range(KT):
        nc.sync.dma_start(wqk_s[:, kt], wqk[:, kt])
        nc.sync.dma_start(xt_s[:, kt, 0:512], xt[:, kt, 0:512])
    nc.sync.dma_start(bqk_s[:], bqk)
    for kt in range(KT):
        nc.sync.dma_start(wv_s[:, kt], wv[:, kt])
    nc.sync.dma_start(bv_s[:], bv)
    nc.sync.dma_start(xt_s[:, :, 512:1024], xt[:, :, 512:1024])
    nc.sync.dma_start(wp_s[:], wp)
    nc.sync.dma_start(xt_s[:, :, 1024:1536], xt[:, :, 1024:1536])
    nc.sync.dma_start(xt_s[:, :, 1536:2048], xt[:, :, 1536:2048])

    qk_t = [consts.tile([128, T], CD, tag=f"q{jt}", name=f"q{jt}")
            for jt in range(2)]
    # kT per head, zero-padded to full 128 partitions: head h occupies rows
    # (h%2)*64..+64, the other 64 rows stay zero so the shared q tile's other
    # head is masked out of the full-K S matmul.
    kz_t = [consts.tile([128, T], CD, tag=f"kz{h}", name=f"kz{h}")
            for h in range(HPC)]
    for h in range(HPC):
        zrow = 64 if h % 2 == 0 else 0
        zap = kz_t[h][zrow:zrow + 64, :]
        if h < 2:
            nc.scalar.memzero(zap)
        else:
            nc.gpsimd.memset(zap, 0.0)
    vext_s = consts.tile([128, TT, HPC * (HS + 1)], CD, tag="vext")
    vext4 = vext_s[:].rearrange("p t (h c) -> p t h c", c=HS + 1)
    nc.gpsimd.memset(vext4[:, :, :, HS], 1.0)  # ones columns -> l in PV
    # one tile per dt-half (heads 0,1 | heads 2,3): keeps proj dt0 matmul
    # deps exact (a single 3D tile degrades to whole-tile ordering)
    yt_t = [consts.tile([128, T], CD, tag=f"yt{dt}", name=f"yt{dt}")
            for dt in range(2)]

    # fixed 128x128 causal triangle tri[j, c] = 1 if j <= c
    tri_s = consts.tile([128, 128], CD, tag="tri")
    nc.vector.memset(tri_s[:], 1.0)
    nc.gpsimd.affine_select(
        out=tri_s[:],
        in_=tri_s[:],
        compare_op=mybir.AluOpType.is_ge,
        fill=0.0,
        base=0,
        channel_multiplier=-1,
        pattern=[[1, 128]],
    )
    tri = tri_s[:]
    bv4 = bv_s[:].rearrange("p (h d) -> p h d", d=HS)

    # --- pools: PSUM = mm(2 banks) + S(2x2) + y(2) = 8 banks exactly ---
    mm_psum = ctx.enter_context(tc.tile_pool(name="mmp", bufs=2, space="PSUM"))
    attn_sp = ctx.enter_context(tc.tile_pool(name="attn_s", bufs=2, space="PSUM"))
    attn_yp = ctx.enter_context(tc.tile_pool(name="attn_y", bufs=2, space="PSUM"))
    pt_pool = ctx.enter_context(tc.tile_pool(name="pt", bufs=3))
    misc = ctx.enter_context(tc.tile_pool(name="misc", bufs=2))
    stage = ctx.enter_context(tc.tile_pool(name="stage", bufs=2))

    # --- work units ---
    def kq_group(tb, jt):
        tsl = slice(tb * 512, (tb + 1) * 512)
        ps = mm_psum.tile([128, 512], F32, tag="mm", name="psqk")
        for kt in range(KT):
            nc.tensor.matmul(
                out=ps[:],
                lhsT=wqk_s[:, kt, jt, :],
                rhs=xt_s[:, kt, tsl],
                start=(kt == 0),
                stop=(kt == KT - 1),
            )
        if jt < 2:  # q
            nc.vector.tensor_scalar_add(
                qk_t[jt][:, tsl], ps[:], bqk_s[:, jt:jt + 1]
            )
        else:  # k -> zero-padded per-head kz
            h2 = 2 * (jt - 2)
            nc.vector.tensor_scalar_add(
                kz_t[h2][0:64, tsl], ps[0:64, :], bqk_s[0:64, jt:jt + 1]
            )
            nc.vector.tensor_scalar_add(
                kz_t[h2 + 1][64:128, tsl], ps[64:128, :], bqk_s[64:128, jt:jt + 1]
            )

    def v_unit(tt):
        def go():
            ps = mm_psum.tile([128, 256], F32, tag="mm", name="psv")
            for kt in range(KT):
                nc.tensor.matmul(
                    out=ps[:],
                    lhsT=xt_s[:, kt, tt * 128:(tt + 1) * 128],
                    rhs=wv_s[:, kt],
                    start=(kt == 0),
                    stop=(kt == KT - 1),
                )
            nc.vector.tensor_add(
                vext4[:, tt, :, 0:HS],
                ps[:].rearrange("p (h d) -> p h d", d=HS),
                bv4,
            )
        return (tt, go)

    def proj_unit(ttp, act_copy=False):
        def go():
            st = stage.tile([128, 1024], CD, tag="st", name="st")
            for eb in range(2):
                psp = mm_psum.tile([128, 512], F32, tag="mm", name="psp")
                for dt in range(2):
                    nc.tensor.matmul(
                        out=psp[:],
                        lhsT=yt_t[dt][:, ttp * 128:(ttp + 1) * 128],
                        rhs=wp_s[:, dt, eb * 512:(eb + 1) * 512],
                        start=(dt == 0),
                        stop=(dt == 1),
                    )
                esl = slice(eb * 512, (eb + 1) * 512)
                # PSUM->SBUF copy rides on ACT when it has exp slack (early
                # i-blocks / drain eb0), keeping the DVE queue short for
                # masks and normalize chains
                if act_copy == "full" or (act_copy and eb == 0):
                    nc.scalar.copy(st[:, esl], psp[:])
                else:
                    nc.vector.tensor_copy(st[:, esl], psp[:])
            nc.sync.dma_start(out[ttp * 128:(ttp + 1) * 128, :], st[:])
        return go

    pend_v = []      # (tt, closure): must emit before PV reading j-tile tt
    pend_kq = []     # (tb, closure): upcoming q/k projections (fillable)
    pend_fill = []   # generic PE filler (previous i-block's projection)

    def run_v_upto(jmax):
        while pend_v and pend_v[0][0] <= jmax:
            pend_v.pop(0)[1]()

    def run_kq_for(tb):
        while pend_kq and pend_kq[0][0] <= tb:
            pend_kq.pop(0)[2]()

    def fill(n=1):
        for _ in range(n):
            if pend_v:
                pend_v.pop(0)[1]()
            elif pend_kq:
                pend_kq.pop(0)[2]()
            elif pend_fill:
                pend_fill.pop(0)()
            else:
                break

    pend_lchain = []  # deferred per-head softmax-normalize chains

    def flush_lchain():
        while pend_lchain:
            pend_lchain.pop(0)()

    def emit_pv(h, ib, grp, offs, ws, cs, pt, psy, njt):
        run_v_upto(grp[-1])
        for gi, j in enumerate(grp):
            nc.tensor.matmul(
                out=psy[:, offs[gi]:512],
                lhsT=vext4[:, j, h, :],
                rhs=pt[:, cs[gi]:cs[gi] + ws[gi]],
                start=(j == 0),
                stop=(j == njt - 1),
            )

    def lchain_unit(h, ib, psy):
        jt_q = h // 2
        row = (h % 2) * 64
        isl = slice(ib * 512, (ib + 1) * 512)
        # the very last chain gates the whole projection drain: let the tile
        # scheduler run it ahead of any queued staging copies on the DVE
        hi = (ib == IB - 1 and h == HPC - 1)

        def go():
            # softmax denominator: row 64 of psy is l = sum_j P.
            # stage the row to SBUF (custom-DVE recip can't read PSUM), take
            # 1/l on the [1,512] row, partition-broadcast on idle gpsimd.
            from contextlib import nullcontext
            with tc.high_priority() if hi else nullcontext():
                lrow = misc.tile([1, 512], F32, tag="lrow")
                nc.vector.tensor_copy(lrow[:], psy[HS:HS + 1, :])
                linv = misc.tile([1, 512], F32, tag="lrow")
                nc.vector.reciprocal_approx_fast(linv[:], lrow[:])
                lbc = misc.tile([64, 512], F32, tag="lbc")
                nc.gpsimd.partition_broadcast(lbc[:], linv[:], channels=64)
                nc.vector.tensor_mul(
                    yt_t[jt_q][row:row + 64, isl], psy[0:HS, :], lbc[:]
                )
        return go

    def attn(ib):
        # S/exp/PV in pairs of 4 j-tiles: one [128, <=2048] contiguous-packed
        # S tile per pair -> a single ACTIVATE covers 4 tiles (halves the ACT
        # instruction count; PSUM matmul writes may cross bank boundaries).
        njt = 4 * ib + 4
        prevs = []  # rolling deferred PVs (depth 2), carried across head
        #             boundaries so a head's last groups never wait their exp

        def pop_emit():
            pv_args, chain_info = prevs.pop(0)
            emit_pv(*pv_args)
            if chain_info is not None:  # this PV closed out a head's psy
                pend_lchain.append(lchain_unit(*chain_info))

        for h in range(HPC):
            jt_q = h // 2
            if h == 2:
                # heads 2,3 need q1/k1: force any still-pending projections
                # of this time block before their S matmuls
                run_kq_for(ib)
            psy = attn_yp.tile([HS + 1, 512], F32, tag="y", name="psy")
            for j0 in range(0, njt, 2):
                grp = (j0, j0 + 1)
                offs = [max(0, 128 * j - 512 * ib) for j in grp]
                ws = [512 - o for o in offs]
                cs = [0, ws[0]]  # narrowed tiles pack contiguously in PSUM
                wflat = ws[0] + ws[1]
                psS = attn_sp.tile([128, 1024], F32, tag="s", name="psS")
                pt = pt_pool.tile([128, 1024], CD, tag="pt", name="pt")
                for gi, j in enumerate(grp):
                    nc.tensor.matmul(
                        out=psS[:, cs[gi]: cs[gi] + ws[gi]],
                        lhsT=kz_t[h][:, j * 128:(j + 1) * 128],
                        rhs=qk_t[jt_q][:, ib * 512 + offs[gi]:(ib + 1) * 512],
                        start=True,
                        stop=True,
                    )
                nc.scalar.activation(
                    out=pt[:, 0:wflat], in_=psS[:, 0:wflat],
                    func=Exp, scale=0.125,
                )
                for gi, j in enumerate(grp):
                    if 128 * j >= 512 * ib:  # diagonal tile -> mask boundary
                        nc.vector.tensor_mul(
                            pt[:, cs[gi]:cs[gi] + 128],
                            pt[:, cs[gi]:cs[gi] + 128],
                            tri,
                        )
                if j0 == 0:
                    # previous head's normalize goes behind this head's first
                    # masks so the PE never waits on the DVE chain
                    flush_lchain()
                fill(1)
                if len(prevs) >= 2:
                    pop_emit()
                prevs.append((
                    (h, ib, grp, offs, ws, cs, pt, psy, njt),
                    (h, ib, psy) if j0 + 2 >= njt else None,
                ))
        fill(1)
        while prevs:
            pop_emit()

    # --- main pipelined loop ---
    # kq(tb0) q0/k0 run up front (heads 0,1 unblocked); q1/k1 and kq(tb+1)
    # ride as filler inside attn so the PE absorbs DMA trickle and the
    # Scalar-engine exp stream never pauses at time-block boundaries.
    for jt in (0, 2, 1, 3):  # q0, k0, q1, k1: S(h0) ready after 2 groups
        kq_group(0, jt)
    for tb in range(IB):
        run_kq_for(tb)
        for tt in range(4 * tb, 4 * tb + 4):
            pend_v.append(v_unit(tt))
        if tb + 1 < IB:
            for jt in (0, 2, 1, 3):
                pend_kq.append(
                    (lambda t, j: (t, j, lambda: kq_group(t, j)))(tb + 1, jt)
                )
        attn(tb)
        for ttp in range(4 * tb, 4 * tb + 4):
            pend_fill.append(proj_unit(ttp, act_copy=(tb != 2)))
    flush_lchain()
    while pend_v or pend_kq or pend_fill:
        fill(1)


def build():
    if "nc" in _CACHE:
        return _CACHE["nc"]
    nc = bacc.Bacc(
        "TRN2", target_bir_lowering=False, debug=False, num_devices=N_CORES
    )
    aps = {
        "xt": nc.dram_tensor("xt", [128, KT, T], CD, kind="ExternalInput").ap(),
        "wqk": nc.dram_tensor("wqk", [128, KT, 4, 128], CD, kind="ExternalInput").ap(),
        "wv": nc.dram_tensor("wv", [128, KT, 256], CD, kind="ExternalInput").ap(),
        "bqk": nc.dram_tensor("bqk", [128, 4], F32, kind="ExternalInput").ap(),
        "bv": nc.dram_tensor("bv", [128, 256], CD, kind="ExternalInput").ap(),
        "wp": nc.dram_tensor("wp", [128, 2, C], CD, kind="ExternalInput").ap(),
        "out": nc.dram_tensor("out", [T, C], CD, kind="ExternalOutput").ap(),
    }
    from contextlib import ExitStack

    with tile.TileContext(nc) as tc:
        with ExitStack() as ctx:
            _emit(nc, tc, ctx, aps)
    nc.compile()
    _CACHE["nc"] = nc
    return nc


def make_in_maps(x, W_attn, b_attn, W_proj, b_proj):
    x = np.asarray(x, dtype=np.float32)
    W_attn = np.asarray(W_attn, dtype=np.float32)
    b_attn = np.asarray(b_attn, dtype=np.float32)
    W_proj = np.asarray(W_proj, dtype=np.float32)
    b_proj = np.asarray(b_proj, dtype=np.float32)

    in_maps = []
    xt_b = {}
    for b in range(B):
        xt = np.ascontiguousarray(x[b].T)  # [C, T]
        xt_b[b] = (
            xt.reshape(KT, 128, T).transpose(1, 0, 2).astype(CD_NP)
        )
    for core in range(N_CORES):
        b = core // 4
        g = core % 4
        fs = slice(256 * g, 256 * g + 256)  # feature cols for this head group
        wq = W_attn[:, fs]
        wk = W_attn[:, C + 256 * g: C + 256 * g + 256]
        wv = W_attn[:, 2 * C + 256 * g: 2 * C + 256 * g + 256]
        wqk = np.concatenate([wq, wk], axis=1)  # [1024, 512]
        bq = b_attn[fs]
        bk = b_attn[C + 256 * g: C + 256 * g + 256]
        bv = b_attn[2 * C + 256 * g: 2 * C + 256 * g + 256]
        in_maps.append({
            "xt": xt_b[b],
            "wqk": np.ascontiguousarray(
                wqk.reshape(KT, 128, 4, 128).transpose(1, 0, 2, 3)
            ).astype(CD_NP),
            "wv": np.ascontiguousarray(
                wv.reshape(KT, 128, 256).transpose(1, 0, 2)
            ).astype(CD_NP),
            "bqk": np.ascontiguousarray(
                np.concatenate([bq, bk]).reshape(4, 128).T
            ).astype(np.float32),
            "bv": np.ascontiguousarray(
                np.broadcast_to(bv[None, :], (128, 256))
            ).astype(CD_NP),
            "wp": np.ascontiguousarray(
                W_proj[fs, :].reshape(2, 128, C).transpose(1, 0, 2)
            ).astype(CD_NP),
        })
    return in_maps, b_proj


def _ensure_ntff_hook():
    """Recreate the missing antenv.axon_hooks NTFF-profile shim (see
    trn_agent_boot/trn_boot.py) so run_bass_kernel_spmd(trace=True) works."""
    import contextlib
    import ctypes
    import types

    try:
        from antenv.axon_hooks import get_axon_ntff_profile_hook  # noqa: F401

        return
    except ImportError:
        pass

    mod = types.ModuleType("antenv.axon_hooks")
    _holder = {"hook": None}
    mod.set_axon_ntff_profile_hook = lambda h: _holder.__setitem__("hook", h)
    mod.get_axon_ntff_profile_hook = lambda: _holder["hook"]
    sys.modules["antenv.axon_hooks"] = mod
    import antenv

    antenv.axon_hooks = mod

    so_path = "/opt/axon/libaxon_pjrt.so"
    if not os.path.exists(so_path):
        return
    lib = ctypes.CDLL(so_path)
    if not hasattr(lib, "axon_start_nrt_profile"):
        return
    lib.axon_start_nrt_profile.argtypes = [
        ctypes.POINTER(ctypes.c_int64),
        ctypes.c_size_t,
    ]
    lib.axon_start_nrt_profile.restype = ctypes.c_int64
    lib.axon_stop_nrt_profile.argtypes = [ctypes.c_char_p]
    lib.axon_stop_nrt_profile.restype = ctypes.c_int64

    @contextlib.contextmanager
    def _hook(output_dir, device_ids):
        import jax

        jax.devices()
        if device_ids:
            ids = (ctypes.c_int64 * len(device_ids))(*device_ids)
            rc = lib.axon_start_nrt_profile(ids, len(device_ids))
        else:
            rc = lib.axon_start_nrt_profile(None, 0)
        if rc != 0:
            raise RuntimeError(f"axon_start_nrt_profile rc={rc}")
        try:
            yield
        finally:
            n = lib.axon_stop_nrt_profile(str(output_dir).encode())
            if n <= 0:
                print(f"ntff profile: rc={n}, nothing written to {output_dir}")

    mod.set_axon_ntff_profile_hook(_hook)


def kernel(x, W_attn, b_attn, W_proj, b_proj):
    global LAST_RESULT
    nc = build()
    in_maps, b_proj_f32 = make_in_maps(x, W_attn, b_attn, W_proj, b_proj)
    trace = os.environ.get("KERNEL_TRACE", "0") == "1"
    if trace:
        _ensure_ntff_hook()
        import concourse.bass_utils as _bu

        _bu.upload_artifacts = lambda tmpdir: f"local://{tmpdir}"
    res = run_bass_kernel_spmd(
        nc, in_maps, core_ids=list(range(N_CORES)), trace=trace
    )
    LAST_RESULT = res
    outs = [res.results[i]["out"].astype(np.float32) for i in range(N_CORES)]
    y = np.empty((B, T, C), dtype=np.float32)
    for b in range(B):
        y[b] = (outs[4 * b] + outs[4 * b + 1] + outs[4 * b + 2]
                + outs[4 * b + 3] + b_proj_f32[None, :])
    return y


# revision 65
# speedup vs baseline: 1.0080x; 1.0080x over previous
"""Causal self-attention (B=2, T=2048, C=1024, nh=16) on 8 TRN2 NeuronCores.

Sharding: core c -> batch b = c//4, head group g = c%4 (4 heads each).
Each core computes QKV projections for its heads, causal attention, and a
partial output projection (W_proj rows for its heads). The four partials
per batch are summed on the host, which also adds b_proj.

Layouts (per core, hardcoded):
  xt   [128, 8, 2048]    x[b].T tiles:  xt[p, kt, t] = x[b, t, kt*128+p]
  wqk  [128, 8, 4, 128]  W_attn q|k cols for this core's heads
  wv   [128, 8, 256]     W_attn v cols (flat 256 feature cols)
  bqk  [128, 4] f32      b_attn q|k (per-partition bias)
  bv   [128, 256] bf16   b_attn v broadcast across partitions (host-side)
  wp   [128, 2, 1024]    W_proj rows for this core's heads
  out  [2048, 1024] bf16 partial (x[b] @ ... for this head group)

Single software-pipelined pass (vs the old 3-phase structure):
  for tb in 0..3:
    q,k projections for time block tb (W-stationary, [feat, t] layout)
    attention for i-block ib=tb:  S^T = k @ q.T -> exp on ACT -> P^T @ [v|1]
      - PE filler inside the S->exp->PV dependency gaps: v projections
        (x @ Wv, natural [t, feat] layout, feeding vext directly -- no DMA
        transpose), the previous i-block's output projection, and the NEXT
        time block's q/k projections. The PE never idles (keeps the 2.4 GHz
        p-state) and the Scalar-engine exp stream is hidden under PE work.
      - PV matmuls are deferred by two groups (rolling across head
        boundaries) so no PV ever waits on its exp; each head's softmax
        normalize chain (1/l + broadcast + scale) is deferred behind the
        next head's masks so the PE never waits on the DVE.
  drain: output projection of the last i-block (PSUM->SBUF copies split
  across ACT and DVE).

DMA: one sync-engine ring, issue order = need order (per-kt wqk/xt-tb0
pairs first so the first matmul starts ~9us in, then wv, xt tb1..3 as
single whole-tensor transfers). Output stores are 2KB-line bf16 tiles;
the host sums the 4 partials per batch and adds b_proj (no bias matmuls
on device).

Baseline from the previous session: 200.3us. This version: ~156us.
"""

import os
import sys

sys.path.insert(0, "/opt/trn_rl_repo")
os.environ.setdefault("MYCRO_LOCAL_CACHE", "1")

import ml_dtypes
import numpy as np

import concourse.bass as bass
import concourse.mybir as mybir
import concourse.tile as tile
from concourse import bacc
from concourse.bass_utils import run_bass_kernel_spmd

B, T, C, NH, HS = 2, 2048, 1024, 16, 64
HPC = 4  # heads per core
N_CORES = 8
KT = C // 128  # 8 contraction tiles over C
TT = T // 128  # 16 tiles over T
IB = T // 512  # 4 i-blocks over T
F32 = mybir.dt.float32

CD = mybir.dt.bfloat16
CD_NP = ml_dtypes.bfloat16

LAST_RESULT = None
_CACHE = {}


def _emit(nc, tc, ctx, aps):
    xt, wqk, wv, bqk, bv, wp, out = (
        aps["xt"], aps["wqk"], aps["wv"], aps["bqk"], aps["bv"], aps["wp"],
        aps["out"],
    )
    Exp = mybir.ActivationFunctionType.Exp

    consts = ctx.enter_context(tc.tile_pool(name="consts", bufs=1))

    # --- persistent SBUF tensors + input DMAs in need-order on one ring ---
    xt_s = consts.tile([128, KT, T], CD, tag="xt")
    wqk_s = consts.tile([128, KT, 4, 128], CD, tag="wqk")
    wv_s = consts.tile([128, KT, 256], CD, tag="wv")
    bqk_s = consts.tile([128, 4], F32, tag="bqk")
    bv_s = consts.tile([128, 256], CD, tag="bv")
    wp_s = consts.tile([128, 2, C], CD, tag="wp")

    # need-order: (wqk, xt-tb0) pairs -> bqk -> wv -> bv -> xt tb1 -> wp ->
    # xt tb2, tb3. The ring drains FIFO at full HBM bandwidth, so the first
    # q-projection chain unblocks ~9us in instead of waiting for everything.
    for kt in range(KT):
        nc.sync.dma_start(wqk_s[:, kt], wqk[:, kt])
        nc.sync.dma_start(xt_s[:, kt, 0:512], xt[:, kt, 0:512])
    nc.sync.dma_start(bqk_s[:], bqk)
    for kt in range(KT):
        nc.sync.dma_start(wv_s[:, kt], wv[:, kt])
    nc.sync.dma_start(bv_s[:], bv)
    nc.sync.dma_start(xt_s[:, :, 512:1024], xt[:, :, 512:1024])
    nc.sync.dma_start(wp_s[:], wp)
    nc.sync.dma_start(xt_s[:, :, 1024:1536], xt[:, :, 1024:1536])
    nc.sync.dma_start(xt_s[:, :, 1536:2048], xt[:, :, 1536:2048])

    qk_t = [consts.tile([128, T], CD, tag=f"q{jt}", name=f"q{jt}")
            for jt in range(2)]
    # kT per head, zero-padded to full 128 partitions: head h occupies rows
    # (h%2)*64..+64, the other 64 rows stay zero so the shared q tile's other
    # head is masked out of the full-K S matmul.
    kz_t = [consts.tile([128, T], CD, tag=f"kz{h}", name=f"kz{h}")
            for h in range(HPC)]
    for h in range(HPC):
        zrow = 64 if h % 2 == 0 else 0
        zap = kz_t[h][zrow:zrow + 64, :]
        if h < 2:
            nc.scalar.memzero(zap)
        else:
            nc.gpsimd.memset(zap, 0.0)
    vext_s = consts.tile([128, TT, HPC * (HS + 1)], CD, tag="vext")
    vext4 = vext_s[:].rearrange("p t (h c) -> p t h c", c=HS + 1)
    nc.gpsimd.memset(vext4[:, :, :, HS], 1.0)  # ones columns -> l in PV
    # one tile per dt-half (heads 0,1 | heads 2,3): keeps proj dt0 matmul
    # deps exact (a single 3D tile degrades to whole-tile ordering)
    yt_t = [consts.tile([128, T], CD, tag=f"yt{dt}", name=f"yt{dt}")
            for dt in range(2)]

    # fixed 128x128 causal triangle tri[j, c] = 1 if j <= c
    tri_s = consts.tile([128, 128], CD, tag="tri")
    nc.vector.memset(tri_s[:], 1.0)
    nc.gpsimd.affine_select(
        out=tri_s[:],
        in_=tri_s[:],
        compare_op=mybir.AluOpType.is_ge,
        fill=0.0,
        base=0,
        channel_multiplier=-1,
        pattern=[[1, 128]],
    )
    tri = tri_s[:]
    bv4 = bv_s[:].rearrange("p (h d) -> p h d", d=HS)

    # --- pools: PSUM = mm(2 banks) + S(2x2) + y(2) = 8 banks exactly ---
    mm_psum = ctx.enter_context(tc.tile_pool(name="mmp", bufs=2, space="PSUM"))
    attn_sp = ctx.enter_context(tc.tile_pool(name="attn_s", bufs=2, space="PSUM"))
    attn_yp = ctx.enter_context(tc.tile_pool(name="attn_y", bufs=2, space="PSUM"))
    pt_pool = ctx.enter_context(tc.tile_pool(name="pt", bufs=3))
    misc = ctx.enter_context(tc.tile_pool(name="misc", bufs=2))
    stage = ctx.enter_context(tc.tile_pool(name="stage", bufs=2))

    # --- work units ---
    def kq_group(tb, jt):
        tsl = slice(tb * 512, (tb + 1) * 512)
        ps = mm_psum.tile([128, 512], F32, tag="mm", name="psqk")
        for kt in range(KT):
            nc.tensor.matmul(
                out=ps[:],
                lhsT=wqk_s[:, kt, jt, :],
                rhs=xt_s[:, kt, tsl],
                start=(kt == 0),
                stop=(kt == KT - 1),
            )
        if jt < 2:  # q
            nc.vector.tensor_scalar_add(
                qk_t[jt][:, tsl], ps[:], bqk_s[:, jt:jt + 1]
            )
        else:  # k -> zero-padded per-head kz
            h2 = 2 * (jt - 2)
            nc.vector.tensor_scalar_add(
                kz_t[h2][0:64, tsl], ps[0:64, :], bqk_s[0:64, jt:jt + 1]
            )
            nc.vector.tensor_scalar_add(
                kz_t[h2 + 1][64:128, tsl], ps[64:128, :], bqk_s[64:128, jt:jt + 1]
            )

    def v_unit(tt):
        def go():
            ps = mm_psum.tile([128, 256], F32, tag="mm", name="psv")
            for kt in range(KT):
                nc.tensor.matmul(
                    out=ps[:],
                    lhsT=xt_s[:, kt, tt * 128:(tt + 1) * 128],
                    rhs=wv_s[:, kt],
                    start=(kt == 0),
                    stop=(kt == KT - 1),
                )
            nc.vector.tensor_add(
                vext4[:, tt, :, 0:HS],
                ps[:].rearrange("p (h d) -> p h d", d=HS),
                bv4,
            )
        return (tt, go)

    def proj_unit(ttp, act_copy=False):
        def go():
            st = stage.tile([128, 1024], CD, tag="st", name="st")
            for eb in range(2):
                psp = mm_psum.tile([128, 512], F32, tag="mm", name="psp")
                for dt in range(2):
                    nc.tensor.matmul(
                        out=psp[:],
                        lhsT=yt_t[dt][:, ttp * 128:(ttp + 1) * 128],
                        rhs=wp_s[:, dt, eb * 512:(eb + 1) * 512],
                        start=(dt == 0),
                        stop=(dt == 1),
                    )
                esl = slice(eb * 512, (eb + 1) * 512)
                # PSUM->SBUF copy rides on ACT when it has exp slack (early
                # i-blocks / drain eb0), keeping the DVE queue short for
                # masks and normalize chains
                if act_copy == "full" or (act_copy and eb == 0):
                    nc.scalar.copy(st[:, esl], psp[:])
                else:
                    nc.vector.tensor_copy(st[:, esl], psp[:])
            nc.sync.dma_start(out[ttp * 128:(ttp + 1) * 128, :], st[:])
        return go

    pend_v = []      # (tt, closure): must emit before PV reading j-tile tt
    pend_kq = []     # (tb, closure): upcoming q/k projections (fillable)
    pend_fill = []   # generic PE filler (previous i-block's projection)

    def run_v_upto(jmax):
        while pend_v and pend_v[0][0] <= jmax:
            pend_v.pop(0)[1]()

    def run_kq_for(tb):
        while pend_kq and pend_kq[0][0] <= tb:
            pend_kq.pop(0)[2]()

    def fill(n=1):
        for _ in range(n):
            if pend_v:
                pend_v.pop(0)[1]()
            elif pend_kq:
                pend_kq.pop(0)[2]()
            elif pend_fill:
                pend_fill.pop(0)()
            else:
                break

    pend_lchain = []  # deferred per-head softmax-normalize chains

    def flush_lchain():
        while pend_lchain:
            pend_lchain.pop(0)()

    def emit_pv(h, ib, grp, offs, ws, cs, pt, psy, njt):
        run_v_upto(grp[-1])
        for gi, j in enumerate(grp):
            nc.tensor.matmul(
                out=psy[:, offs[gi]:512],
                lhsT=vext4[:, j, h, :],
                rhs=pt[:, cs[gi]:cs[gi] + ws[gi]],
                start=(j == 0),
                stop=(j == njt - 1),
            )

    def lchain_unit(h, ib, psy):
        jt_q = h // 2
        row = (h % 2) * 64
        isl = slice(ib * 512, (ib + 1) * 512)
        def go():
            # softmax denominator: row 64 of psy is l = sum_j P.
            # stage the row to SBUF (custom-DVE recip can't read PSUM), take
            # 1/l on the [1,512] row, partition-broadcast on idle gpsimd.
            lrow = misc.tile([1, 512], F32, tag="lrow")
            nc.vector.tensor_copy(lrow[:], psy[HS:HS + 1, :])
            linv = misc.tile([1, 512], F32, tag="lrow")
            nc.vector.reciprocal_approx_fast(linv[:], lrow[:])
            lbc = misc.tile([64, 512], F32, tag="lbc")
            nc.gpsimd.partition_broadcast(lbc[:], linv[:], channels=64)
            nc.vector.tensor_mul(
                yt_t[jt_q][row:row + 64, isl], psy[0:HS, :], lbc[:]
            )
        return go

    def attn(ib):
        # S/exp/PV in pairs of 4 j-tiles: one [128, <=2048] contiguous-packed
        # S tile per pair -> a single ACTIVATE covers 4 tiles (halves the ACT
        # instruction count; PSUM matmul writes may cross bank boundaries).
        njt = 4 * ib + 4
        prevs = []  # rolling deferred PVs (depth 2), carried across head
        #             boundaries so a head's last groups never wait their exp

        def pop_emit():
            pv_args, chain_info = prevs.pop(0)
            emit_pv(*pv_args)
            if chain_info is not None:  # this PV closed out a head's psy
                pend_lchain.append(lchain_unit(*chain_info))

        for h in range(HPC):
            jt_q = h // 2
            if h == 2:
                # heads 2,3 need q1/k1: force any still-pending projections
                # of this time block before their S matmuls
                run_kq_for(ib)
            psy = attn_yp.tile([HS + 1, 512], F32, tag="y", name="psy")
            for j0 in range(0, njt, 2):
                grp = (j0, j0 + 1)
                offs = [max(0, 128 * j - 512 * ib) for j in grp]
                ws = [512 - o for o in offs]
                cs = [0, ws[0]]  # narrowed tiles pack contiguously in PSUM
                wflat = ws[0] + ws[1]
                psS = attn_sp.tile([128, 1024], F32, tag="s", name="psS")
                pt = pt_pool.tile([128, 1024], CD, tag="pt", name="pt")
                for gi, j in enumerate(grp):
                    nc.tensor.matmul(
                        out=psS[:, cs[gi]: cs[gi] + ws[gi]],
                        lhsT=kz_t[h][:, j * 128:(j + 1) * 128],
                        rhs=qk_t[jt_q][:, ib * 512 + offs[gi]:(ib + 1) * 512],
                        start=True,
                        stop=True,
                    )
                nc.scalar.activation(
                    out=pt[:, 0:wflat], in_=psS[:, 0:wflat],
                    func=Exp, scale=0.125,
                )
                for gi, j in enumerate(grp):
                    if 128 * j >= 512 * ib:  # diagonal tile -> mask boundary
                        nc.vector.tensor_mul(
                            pt[:, cs[gi]:cs[gi] + 128],
                            pt[:, cs[gi]:cs[gi] + 128],
                            tri,
                        )
                if j0 == 0:
                    # previous head's normalize goes behind this head's first
                    # masks so the PE never waits on the DVE chain
                    flush_lchain()
                fill(1)
                if len(prevs) >= 2:
                    pop_emit()
                prevs.append((
                    (h, ib, grp, offs, ws, cs, pt, psy, njt),
                    (h, ib, psy) if j0 + 2 >= njt else None,
                ))
        fill(1)
        while prevs:
            pop_emit()

    # --- main pipelined loop ---
    # kq(tb0) q0/k0 run up front (heads 0,1 unblocked); q1/k1 and kq(tb+1)
    # ride as filler inside attn so the PE absorbs DMA trickle and the
    # Scalar-engine exp stream never pauses at time-block boundaries.
    for jt in (0, 2, 1, 3):  # q0, k0, q1, k1: S(h0) ready after 2 groups
        kq_group(0, jt)
    for tb in range(IB):
        run_kq_for(tb)
        for tt in range(4 * tb, 4 * tb + 4):
            pend_v.append(v_unit(tt))
        if tb + 1 < IB:
            for jt in (0, 2, 1, 3):
                pend_kq.append(
                    (lambda t, j: (t, j, lambda: kq_group(t, j)))(tb + 1, jt)
                )
        attn(tb)
        for ttp in range(4 * tb, 4 * tb + 4):
            pend_fill.append(proj_unit(ttp, act_copy=(tb != 2)))
    flush_lchain()
    while pend_v or pend_kq or pend_fill:
        fill(1)


def build():
    if "nc" in _CACHE:
        return _CACHE["nc"]
    nc = bacc.Bacc(
        "TRN2", target_bir_lowering=False, debug=False, num_devices=N_CORES
    )
    aps = {
        "xt": nc.dram_tensor("xt", [128, KT, T], CD, kind="ExternalInput").ap(),
        "wqk": nc.dram_tensor("wqk", [128, KT, 4, 128], CD, kind="ExternalInput").ap(),
        "wv": nc.dram_tensor("wv", [128, KT, 256], CD, kind="ExternalInput").ap(),
        "bqk": nc.dram_tensor("bqk", [128, 4], F32, kind="ExternalInput").ap(),
        "bv": nc.dram_tensor("bv", [128, 256], CD, kind="ExternalInput").ap(),
        "wp": nc.dram_tensor("wp", [128, 2, C], CD, kind="ExternalInput").ap(),
        "out": nc.dram_tensor("out", [T, C], CD, kind="ExternalOutput").ap(),
    }
    from contextlib import ExitStack

    with tile.TileContext(nc) as tc:
        with ExitStack() as ctx:
            _emit(nc, tc, ctx, aps)
    nc.compile()
    _CACHE["nc"] = nc
    return nc


def make_in_maps(x, W_attn, b_attn, W_proj, b_proj):
    x = np.asarray(x, dtype=np.float32)
    W_attn = np.asarray(W_attn, dtype=np.float32)
    b_attn = np.asarray(b_attn, dtype=np.float32)
    W_proj = np.asarray(W_proj, dtype=np.float32)
    b_proj = np.asarray(b_proj, dtype=np.float32)

    in_maps = []
    xt_b = {}
    for b in range(B):
        xt = np.ascontiguousarray(x[b].T)  # [C, T]
        xt_b[b] = (
            xt.reshape(KT, 128, T).transpose(1, 0, 2).astype(CD_NP)
        )
    for core in range(N_CORES):
        b = core // 4
        g = core % 4
        fs = slice(256 * g, 256 * g + 256)  # feature cols for this head group
        wq = W_attn[:, fs]
        wk = W_attn[:, C + 256 * g: C + 256 * g + 256]
        wv = W_attn[:, 2 * C + 256 * g: 2 * C + 256 * g + 256]
        wqk = np.concatenate([wq, wk], axis=1)  # [1024, 512]
        bq = b_attn[fs]
        bk = b_attn[C + 256 * g: C + 256 * g + 256]
        bv = b_attn[2 * C + 256 * g: 2 * C + 256 * g + 256]
        in_maps.append({
            "xt": xt_b[b],
            "wqk": np.ascontiguousarray(
                wqk.reshape(KT, 128, 4, 128).transpose(1, 0, 2, 3)
            ).astype(CD_NP),
            "wv": np.ascontiguousarray(
                wv.reshape(KT, 128, 256).transpose(1, 0, 2)
            ).astype(CD_NP),
            "bqk": np.ascontiguousarray(
                np.concatenate([bq, bk]).reshape(4, 128).T
            ).astype(np.float32),
            "bv": np.ascontiguousarray(
                np.broadcast_to(bv[None, :], (128, 256))
            ).astype(CD_NP),
            "wp": np.ascontiguousarray(
                W_proj[fs, :].reshape(2, 128, C).transpose(1, 0, 2)
            ).astype(CD_NP),
        })
    return in_maps, b_proj


def _ensure_ntff_hook():
    """Recreate the missing antenv.axon_hooks NTFF-profile shim (see
    trn_agent_boot/trn_boot.py) so run_bass_kernel_spmd(trace=True) works."""
    import contextlib
    import ctypes
    import types

    try:
        from antenv.axon_hooks import get_axon_ntff_profile_hook  # noqa: F401

        return
    except ImportError:
        pass

    mod = types.ModuleType("antenv.axon_hooks")
    _holder = {"hook": None}
    mod.set_axon_ntff_profile_hook = lambda h: _holder.__setitem__("hook", h)
    mod.get_axon_ntff_profile_hook = lambda: _holder["hook"]
    sys.modules["antenv.axon_hooks"] = mod
    import antenv

    antenv.axon_hooks = mod

    so_path = "/opt/axon/libaxon_pjrt.so"
    if not os.path.exists(so_path):
        return
    lib = ctypes.CDLL(so_path)
    if not hasattr(lib, "axon_start_nrt_profile"):
        return
    lib.axon_start_nrt_profile.argtypes = [
        ctypes.POINTER(ctypes.c_int64),
        ctypes.c_size_t,
    ]
    lib.axon_start_nrt_profile.restype = ctypes.c_int64
    lib.axon_stop_nrt_profile.argtypes = [ctypes.c_char_p]
    lib.axon_stop_nrt_profile.restype = ctypes.c_int64

    @contextlib.contextmanager
    def _hook(output_dir, device_ids):
        import jax

        jax.devices()
        if device_ids:
            ids = (ctypes.c_int64 * len(device_ids))(*device_ids)
            rc = lib.axon_start_nrt_profile(ids, len(device_ids))
        else:
            rc = lib.axon_start_nrt_profile(None, 0)
        if rc != 0:
            raise RuntimeError(f"axon_start_nrt_profile rc={rc}")
        try:
            yield
        finally:
            n = lib.axon_stop_nrt_profile(str(output_dir).encode())
            if n <= 0:
                print(f"ntff profile: rc={n}, nothing written to {output_dir}")

    mod.set_axon_ntff_profile_hook(_hook)


def kernel(x, W_attn, b_attn, W_proj, b_proj):
    global LAST_RESULT
    nc = build()
    in_maps, b_proj_f32 = make_in_maps(x, W_attn, b_attn, W_proj, b_proj)
    trace = os.environ.get("KERNEL_TRACE", "0") == "1"
    if trace:
        _ensure_ntff_hook()
        import concourse.bass_utils as _bu

        _bu.upload_artifacts = lambda tmpdir: f"local://{tmpdir}"
    res = run_bass_kernel_spmd(
        nc, in_maps, core_ids=list(range(N_CORES)), trace=trace
    )
    LAST_RESULT = res
    outs = [res.results[i]["out"].astype(np.float32) for i in range(N_CORES)]
    y = np.empty((B, T, C), dtype=np.float32)
    for b in range(B):
        y[b] = (outs[4 * b] + outs[4 * b + 1] + outs[4 * b + 2]
                + outs[4 * b + 3] + b_proj_f32[None, :])
    return y
